# revision 19
# baseline (speedup 1.0000x reference)
"""GCNCritic forward kernel for Trainium2 (Bass/Tile), 8-core data-parallel.

Math collapse: the reference GCN runs on fully-connected 16-node graphs with
self-loops, so for every node i in a sample, agg_i + h_i = sum_j h_j — i.e.
each GCN layer's output is constant across the 16 nodes of a sample.  The two
GCN layers + global_mean_pool therefore reduce to per-sample (B-sized)
matmuls on the per-sample mean of x = relu(obs @ W_pre + b_pre):

    xm = mean_nodes(relu(obs @ W_pre + b_pre))            # [B, HID]
    x1 = relu(xm @ W_gcn0 + b_gcn0)                       # [B, HID]
    x2 = relu(x1 @ W_gcn1 + b_gcn1)                       # [B, HID]
    g  = relu(x2 @ W_post + b_post)                       # [B, GE]
    gz = g @ W1[:GE] + b1                                 # [B, F1]
    loc = relu(obs @ W_loc + b_loc)                       # [B*n, LE]
    z1 = relu(loc @ W1[GE:] + gz[sample])                 # [B*n, F1]
    z2 = relu(z1 @ W2 + b2)                               # [B*n, F2]
    q  = z2 @ W3 + b3                                     # [B*n, 8]

Sharding: batch (2048 samples) split across 8 NeuronCores, 256 samples
(4096 nodes) per core; weights replicated.  All activations are kept
feature-on-partitions ("transposed"), so every weight matrix is consumed as
lhsT in its natural [K, M] layout and no transposes are needed in the chain.
Only the initial obs tiles are PE-transposed; q is produced transposed
([8, rows]) and un-transposed host-side.  Matmuls run in float32r
(full-rate fp32 PE mode).  All weights/biases/identity ship as one packed
[128, PACK_COLS] tensor -> single DMA.
"""

import numpy as np

import concourse.bass as bass
import concourse.mybir as mybir
import concourse.tile as tile
from concourse.bass import ts
from concourse.bass_utils import run_bass_kernel_spmd

OBS = 128
N_AGENT = 16
HID = 128
GE = 256
LE = 256
F1 = 512
F2 = 512
NA = 8
B = 2048
NCORES = 8
BS = B // NCORES            # 256 samples per core
R = BS * N_AGENT            # 4096 rows (nodes) per core
RT = 512                    # rows per tile
NT = R // RT                # 8 row tiles
SPT = RT // N_AGENT         # 32 samples per row tile

F32 = mybir.dt.float32
F32R = mybir.dt.float32r
RELU = mybir.ActivationFunctionType.Relu

# packed-constants column layout (see _pack_weights)
C_WPRE = 0
C_WG0 = 128
C_WG1 = 256
C_WPOST = 384
C_WLOC = 640
C_W1 = 896
C_W2 = 2944
C_W3 = 4992
C_BPRE = 5024
C_BG0 = 5025
C_BG1 = 5026
C_BPOST = 5027
C_BLOC = 5029
C_B1 = 5031
C_B2 = 5035
C_B3 = 5039
C_IDENT = 5040
PACK_COLS = 5168


def _pack_weights(i):
    pk = np.zeros((128, PACK_COLS), np.float32)
    pk[:, C_WPRE:C_WPRE + 128] = i["W_pre"]
    pk[:, C_WG0:C_WG0 + 128] = i["W_gcn"][0]
    pk[:, C_WG1:C_WG1 + 128] = i["W_gcn"][1]
    pk[:, C_WPOST:C_WPOST + 256] = i["W_post"]
    pk[:, C_WLOC:C_WLOC + 256] = i["W_loc"]
    for o in range(4):
        pk[:, C_W1 + o * F1:C_W1 + (o + 1) * F1] = i["W1"][o * 128:(o + 1) * 128]
        pk[:, C_W2 + o * F2:C_W2 + (o + 1) * F2] = i["W2"][o * 128:(o + 1) * 128]
        pk[:, C_W3 + o * NA:C_W3 + (o + 1) * NA] = i["W3"][o * 128:(o + 1) * 128]
    pk[:, C_BPRE] = i["b_pre"]
    pk[:, C_BG0] = i["b_gcn"][0]
    pk[:, C_BG1] = i["b_gcn"][1]
    pk[:, C_BPOST:C_BPOST + 2] = i["b_post"].reshape(2, 128).T
    pk[:, C_BLOC:C_BLOC + 2] = i["b_loc"].reshape(2, 128).T
    pk[:, C_B1:C_B1 + 4] = i["b1"].reshape(4, 128).T
    pk[:, C_B2:C_B2 + 4] = i["b2"].reshape(4, 128).T
    pk[:NA, C_B3] = i["b3"]
    pk[:, C_IDENT:C_IDENT + 128] = np.eye(128, dtype=np.float32)
    return pk


def _build():
    nc = bass.Bass("TRN2", target_bir_lowering=False, debug=False)

    obs_h = nc.dram_tensor("obs", [OBS, R], F32R, kind="ExternalInput")
    wpack_h = nc.dram_tensor("wpack", [128, PACK_COLS], F32R, kind="ExternalInput")
    out_h = nc.dram_tensor("out", [NA, R], F32, kind="ExternalOutput")

    with tile.TileContext(nc) as tc:
        with (
            tc.tile_pool(name="consts", bufs=1) as consts,
            tc.tile_pool(name="persist", bufs=1) as persist,
            tc.tile_pool(name="work", bufs=4) as work,
            tc.tile_pool(name="zwork", bufs=3) as zwork,
            tc.tile_pool(name="ps", bufs=8, space="PSUM") as psp,
        ):
            def ptile():
                return psp.tile([128, 512], F32, tag="ps", name="ps")

            # ---- constants: 3 DMAs so phase A's deps (ident/biases/W_pre/
            # W_loc) land quickly while the big W1/W2 block streams in ----
            wp = consts.tile([128, PACK_COLS], F32R, tag="wp")
            nc.sync.dma_start(wp[:, C_W3:], wpack_h[:, C_W3:])
            nc.sync.dma_start(wp[:, :C_W1], wpack_h[:, :C_W1])
            nc.sync.dma_start(wp[:, C_W1:C_W3], wpack_h[:, C_W1:C_W3])

            def wslice(c0, n):
                return wp[:, c0:c0 + n]

            def bias(c0):
                return wp[:, c0:c0 + 1].bitcast(F32)

            ident = wslice(C_IDENT, 128).bitcast(F32)

            # ---- persistent activations ----
            locT = persist.tile([128, 2, NT, RT], F32R, tag="locT")   # loc^T
            xsum = persist.tile([128, BS], F32R, tag="xsum")          # per-sample sums
            gz = persist.tile([128, 4, BS], F32, tag="gz")            # (g @ W1a + b1)^T
            qacc = persist.tile([NA, R], F32, tag="qacc")             # q^T accumulator

            # ---- phase A: per row-tile (obs arrives pre-transposed) ----
            for t in range(NT):
                obsT = work.tile([128, RT], F32R, tag="obsT", bufs=8)
                nc.sync.dma_start(obsT, obs_h[:, ts(t, RT)])

                x_ps = ptile()
                nc.tensor.matmul(
                    x_ps, wslice(C_WPRE, 128), obsT, start=True, stop=True
                )
                xT = work.tile([128, RT], F32R, tag="xT")
                nc.scalar.activation(xT, x_ps, RELU, bias=bias(C_BPRE))

                for m in range(2):
                    l_ps = ptile()
                    nc.tensor.matmul(
                        l_ps, wp[:, C_WLOC + m * 128:C_WLOC + (m + 1) * 128], obsT,
                        start=True, stop=True,
                    )
                    nc.scalar.activation(
                        locT[:, m, t, :], l_ps, RELU, bias=bias(C_BLOC + m)
                    )

                with nc.allow_low_precision(reason="float32r ~ fp32; 16-elem sum"):
                    nc.vector.tensor_reduce(
                        xsum[:, ts(t, SPT)],
                        xT.rearrange("p (s k) -> p s k", k=N_AGENT),
                        axis=mybir.AxisListType.X,
                        op=mybir.AluOpType.add,
                    )

            # ---- phase B: per-sample chain (256 samples at once) ----
            x1_ps = ptile()
            nc.tensor.matmul(
                x1_ps[:, :BS], wslice(C_WG0, 128), xsum, start=True, stop=True
            )
            x1 = work.tile([128, BS], F32R, tag="x1")
            nc.scalar.activation(
                x1, x1_ps[:, :BS], RELU, bias=bias(C_BG0), scale=1.0 / N_AGENT
            )

            x2_ps = ptile()
            nc.tensor.matmul(
                x2_ps[:, :BS], wslice(C_WG1, 128), x1, start=True, stop=True
            )
            x2 = work.tile([128, BS], F32R, tag="x2")
            nc.scalar.activation(x2, x2_ps[:, :BS], RELU, bias=bias(C_BG1))

            g = work.tile([128, 2, BS], F32R, tag="g")
            for m in range(2):
                g_ps = ptile()
                nc.tensor.matmul(
                    g_ps[:, :BS], wp[:, C_WPOST + m * 128:C_WPOST + (m + 1) * 128],
                    x2, start=True, stop=True,
                )
                nc.scalar.activation(
                    g[:, m, :], g_ps[:, :BS], RELU, bias=bias(C_BPOST + m)
                )

            for m in range(4):
                gz_ps = ptile()
                nc.tensor.matmul(
                    gz_ps[:, :BS],
                    wp[:, C_W1 + 0 * F1 + m * 128:C_W1 + 0 * F1 + (m + 1) * 128],
                    g[:, 0, :], start=True, stop=False,
                )
                nc.tensor.matmul(
                    gz_ps[:, :BS],
                    wp[:, C_W1 + 1 * F1 + m * 128:C_W1 + 1 * F1 + (m + 1) * 128],
                    g[:, 1, :], start=False, stop=True,
                )
                nc.vector.tensor_scalar_add(
                    gz[:, m, :], gz_ps[:, :BS], bias(C_B1 + m)
                )

            # ---- phase C: per row-tile: z1 -> z2 -> q^T ----
            for t in range(NT):
                z1 = zwork.tile([128, 4, RT], F32R, tag="z1")
                for m in range(4):
                    z_ps = ptile()
                    nc.tensor.matmul(
                        z_ps,
                        wp[:, C_W1 + 2 * F1 + m * 128:C_W1 + 2 * F1 + (m + 1) * 128],
                        locT[:, 0, t, :], start=True, stop=False,
                    )
                    nc.tensor.matmul(
                        z_ps,
                        wp[:, C_W1 + 3 * F1 + m * 128:C_W1 + 3 * F1 + (m + 1) * 128],
                        locT[:, 1, t, :], start=False, stop=True,
                    )
                    nc.vector.tensor_add(
                        z1[:, m, :].rearrange("p (s k) -> p s k", k=N_AGENT),
                        z_ps.rearrange("p (s k) -> p s k", k=N_AGENT),
                        gz[:, m, ts(t, SPT)][:, :, None].to_broadcast(
                            [128, SPT, N_AGENT]
                        ),
                    )
                    nc.gpsimd.tensor_scalar_max(z1[:, m, :], z1[:, m, :], 0.0)

                z2 = zwork.tile([128, 4, RT], F32R, tag="z2")
                for m in range(4):
                    z_ps = ptile()
                    for k in range(4):
                        nc.tensor.matmul(
                            z_ps,
                            wp[:, C_W2 + k * F2 + m * 128:C_W2 + k * F2 + (m + 1) * 128],
                            z1[:, k, :], start=(k == 0), stop=(k == 3),
                        )
                    nc.scalar.activation(
                        z2[:, m, :], z_ps, RELU, bias=bias(C_B2 + m)
                    )

                q_ps = ptile()
                for k in range(4):
                    nc.tensor.matmul(
                        q_ps[:NA, :], wp[:, C_W3 + k * NA:C_W3 + (k + 1) * NA],
                        z2[:, k, :], start=(k == 0), stop=(k == 3),
                    )
                nc.vector.tensor_scalar_add(
                    qacc[:, ts(t, RT)], q_ps[:NA, :], wp[:NA, C_B3:C_B3 + 1].bitcast(F32)
                )
                if t == NT // 2 - 1:
                    nc.sync.dma_start(out_h[:, :R // 2], qacc[:, :R // 2])
                elif t == NT - 1:
                    nc.sync.dma_start(out_h[:, R // 2:], qacc[:, R // 2:])

    _split_waits(nc)
    return nc


def _split_waits(nc):
    # walrus accepts only one sync-wait per instruction in this build; move
    # extra waits onto same-engine sequencer nops placed immediately before
    # the instruction (program order on the engine's queue, so semantics are
    # identical).
    for blk in nc.m.functions[0].blocks:
        new = []
        for inst in blk.instructions:
            if inst.sync_info is not None:
                w = list(inst.sync_info.on_wait)
                if len(w) > 1:
                    for wx in w[:-1]:
                        new.append(
                            mybir.InstNoOp(
                                name=nc.get_next_instruction_name(),
                                engine=inst.engine,
                                sync_info=mybir.SyncInfo(
                                    on_wait=[wx], on_update=[]
                                ),
                                bass_nofuse=True,
                            )
                        )
                    inst.sync_info.on_wait = [w[-1]]
            new.append(inst)
        blk.instructions[:] = new


_CACHE = {}


def _get_nc():
    if "nc" not in _CACHE:
        _CACHE["nc"] = _build()
    return _CACHE["nc"]


def kernel(trace=False, **inputs):
    obs_j = np.ascontiguousarray(np.asarray(inputs["obs_j"], dtype=np.float32))
    np_in = {
        k: np.asarray(v, dtype=np.float32)
        for k, v in inputs.items()
        if k != "obs_j"
    }
    pack = np.ascontiguousarray(_pack_weights(np_in))
    nc = _get_nc()
    in_maps = []
    for c in range(NCORES):
        in_maps.append({
            "obs": np.ascontiguousarray(obs_j[c * BS:(c + 1) * BS].reshape(R, OBS).T),
            "wpack": pack,
        })
    res = run_bass_kernel_spmd(
        nc, in_maps, core_ids=list(range(NCORES)), trace=trace
    )
    out = np.concatenate([r["out"] for r in res.results], axis=1)  # [NA, B*n]
    q = np.ascontiguousarray(out.T).reshape(B, N_AGENT, NA)
    if trace:
        return q, res
    return q
